# revision 7
# baseline (speedup 1.0000x reference)
"""Trainium2 Bass kernel for nn_GCN_15590731285230 (v4).

Rig model (measured, see mb*.py): non-PE instructions effectively execute
one-at-a-time, blocking on their full access latency:
  * PE matmuls with a loaded stationary: ~2us (hw-decoded, pipelined).
  * Stationary change: ~54us.
  * DVE op on SBUF: ~15-40us regardless of size (up to [128, 32768]).
  * DVE op reading PSUM: ~190us ([128,2048]) / ~230us ([128,4096]).
  * Act op reading PSUM: ~290-550us. gpsimd cannot touch PSUM.
  * Interleaved multi-bank PSUM accumulation (start=False revisiting a
    bank after others): ~66us per matmul -> avoided entirely.
  * DMA cannot read PSUM (bass-level assert).

So: minimize the COUNT of PSUM-reading ops (drain [128,4096] = 2 chunks
per op), keep everything else on cheap SBUF DVE ops, and accept the
16+16 stationary swaps (~54us each) as the PE floor.

v4 structure:
  * pass A (S^T): 16 chunk-rounds into alternating halves of ONE
    [128,4096] psum tile; ONE DVE drain per 2 chunks (8 drains).
  * softmax: strided max reduce + gpsimd all-reduce + sub + Schraudolph
    exp (DVE bit trick) + strided sum reduce + gpsimd all-reduce + recip.
  * r0 DRAM bounce DMAs issued right after exp (overlap the Z pass).
  * Z^T: same psum ping-pong; ONE [128,4096] DVE add per 2 chunks into a
    double-wide accumulator (even chunks left, odd chunks right), merged
    + normalized at the end.
  * h1^T = relu(w1^T Zn + x^T); tail computes row 0 of layer 2 only.

Per batch b (core b):
    R  = softmax(x wr x^T, axis=-1);  h1 = relu(R x w1 + x)
    out_b = relu(R[0,:] @ h1 @ w2 + h1[0,:])
"""

import sys

if "/opt/trn_rl_repo" not in sys.path:
    sys.path.insert(0, "/opt/trn_rl_repo")

from contextlib import ExitStack

import numpy as np

import concourse.bacc as bacc
import concourse.bass as bass
import concourse.bass_isa as bass_isa
import concourse.mybir as mybir
import concourse.tile as tile
from concourse.bass_utils import run_bass_kernel_spmd

P = 128
D = 128
B = 8
F32 = mybir.dt.float32
AF = mybir.ActivationFunctionType
AX = mybir.AxisListType
ALU = mybir.AluOpType
ROP = bass_isa.ReduceOp


def _bcast_free(ap, count):
    """Insert a stride-0 dim of size `count` after the partition dim."""
    return bass.AP(tensor=ap.tensor, offset=ap.offset,
                   ap=[list(ap.ap[0]), [0, count]] + [list(d) for d in ap.ap[1:]])


def build_kernel(n=2048, repeat=1, dbg_stage=None, drain_eng="vector"):
    nt = n // P              # m chunks (16)
    w5 = min(512, n)         # matmul moving-operand width
    nc = bacc.Bacc()
    x_d = nc.dram_tensor("x", [n, D], F32, kind="ExternalInput")
    wall_d = nc.dram_tensor("wall", [3 * D, D], F32, kind="ExternalInput")
    out_d = nc.dram_tensor("out", [1, D], F32, kind="ExternalOutput")
    r0_d = nc.dram_tensor("r0scratch", [1, n], F32, kind="Internal")
    dbg_d = (nc.dram_tensor("dbg", [P, n], F32, kind="ExternalOutput")
             if dbg_stage else None)

    with tile.TileContext(nc) as tc, ExitStack() as ctx:
        sg = ctx.enter_context(tc.tile_pool(name="sg", bufs=1))
        scr = ctx.enter_context(tc.tile_pool(name="scr", bufs=1))
        bb = ctx.enter_context(tc.tile_pool(name="bb", bufs=1))
        st = ctx.enter_context(tc.tile_pool(name="st", bufs=1))
        ps = ctx.enter_context(tc.tile_pool(name="ps", bufs=1, space="PSUM"))

        for _rep in range(repeat):
            dr_copy = (nc.scalar.copy if drain_eng == "scalar"
                       else nc.vector.tensor_copy)

            # one DMA for all three weights: wsb[p, k, :] = w_k[p, :]
            wsb = sg.tile([P, 3, P], F32, tag="wsb")
            nc.sync.dma_start(wsb, wall_d[:].rearrange("(k p) f -> p k f", p=P))
            wr_sb = wsb[:, 0, :]
            w1_sb = wsb[:, 1, :]
            w2_sb = wsb[:, 2, :]

            # natural-enumeration chunked x: xnat[p, t, f] = x[128 t + p, f]
            xnat = sg.tile([P, nt, P], F32, tag="xnat")
            nc.sync.dma_start(xnat, x_d[:].rearrange("(t p) f -> p t f", p=P))
            # transposed x via DMA gather: xT[f, m] = x[m, f]
            xT = sg.tile([P, n], F32, tag="xT")
            nc.sync.dma_start(xT, x_d[:].rearrange("n f -> f n"))

            L = {None: 99, "xT": 1, "passA": 2, "softmax": 3,
                 "z": 4, "h1": 5, "r0": 6, "fin": 8}[dbg_stage]
            if L == 1:
                nc.sync.dma_start(dbg_d[:], xT)
                continue

            # single full-PSUM tile; halves ping-pong inside each pass
            pfull = ps.tile([P, 2 * n], F32, tag="pfull")

            # yT = (x wr)^T : yT[g, m] = sum_f wr[f, g] xT[f, m]
            yT = sg.tile([P, n], F32, tag="yT")
            for j in range(0, n, w5):
                nc.tensor.matmul(pfull[:, j:j + w5], lhsT=wr_sb,
                                 rhs=xT[:, j:j + w5],
                                 start=True, stop=True)
            dr_copy(yT, pfull[:, 0:n])

            # ---- pass A: ST[m, n] = S[n, m] with m = 128 t + p ----
            # chunk g fills half (g%2); one [128, 2n] drain per 2 chunks.
            stall = sg.tile([P, nt, n], F32, tag="stall")
            stall_flat = stall.rearrange("p t n -> p (t n)")
            for g in range(nt):
                base = (g % 2) * n
                for j in range(0, n, w5):
                    nc.tensor.matmul(
                        pfull[:, base + j:base + j + w5],
                        lhsT=xT[:, g * P:(g + 1) * P],
                        rhs=yT[:, j:j + w5],
                        start=True, stop=True)
                if g % 2 == 1:
                    dr_copy(stall_flat[:, (g - 1) * n:(g + 1) * n], pfull)

            if L == 2:
                nc.sync.dma_start(dbg_d[:], stall[:, 0, :])
                continue

            # ---- softmax over m (partitions x chunks), per column n ----
            mx_pt = scr.tile([P, n], F32, tag="scr")
            nc.vector.tensor_reduce(mx_pt, stall.rearrange("p t n -> p n t"),
                                    axis=AX.X, op=ALU.max)
            mtile = bb.tile([P, n], F32, tag="bb")
            nc.gpsimd.partition_all_reduce(mtile, mx_pt, channels=P,
                                           reduce_op=ROP.max)
            nc.vector.tensor_sub(stall_flat, stall_flat,
                                 _bcast_free(mtile[:], nt))
            # exp via Schraudolph bit trick on DVE (Act-engine Exp costs
            # ~5.3 ms for this tensor on this rig; this chain ~0.1 ms):
            #   bits = int32(A * max(u, -87) + B);  E = bitcast_fp32(bits)
            # A = 2^23/ln2, B = 127*2^23 - 486411 (RMS-optimal bias,
            # ~1.5% rms weight error; softmax normalization absorbs most).
            nc.vector.tensor_scalar(
                stall_flat, stall_flat, -87.0,
                scalar2=12102203.161561485,
                op0=ALU.max, op1=ALU.mult)
            nc.vector.tensor_scalar_add(stall_flat, stall_flat, 1064866805.0)
            nc.vector.tensor_copy(stall_flat.bitcast(mybir.dt.int32),
                                  stall_flat)

            # r0 bounce hoisted: runs on DMA queues, overlaps the Z pass.
            # r0 (unnormalized) = exp'd scores at n=0: stall[p, t, 0] written
            # to DRAM at natural position m = 128 t + p, then reloaded flat.
            nc.sync.dma_start(
                r0_d[:].rearrange("o (t p) -> p t o", p=P),
                stall[:, :, 0:1])
            r0row = scr.tile([1, n], F32, tag="r0row")
            nc.sync.dma_start(r0row, r0_d[:])

            etsum = scr.tile([P, n], F32, tag="scr")
            nc.vector.tensor_reduce(etsum, stall.rearrange("p t n -> p n t"),
                                    axis=AX.X, op=ALU.add)
            stile = bb.tile([P, n], F32, tag="bb")
            nc.gpsimd.partition_all_reduce(stile, etsum, channels=P,
                                           reduce_op=ROP.add)
            nc.vector.reciprocal(stile, stile)       # 1/s, all partitions

            if L == 3:
                nc.sync.dma_start(dbg_d[:], stile)
                continue

            # ---- Z^T: even chunks sum into zw[:, 0:n], odd into zw[:, n:] ----
            zw = sg.tile([P, 2 * n], F32, tag="zw")
            for t in range(nt):
                base = (t % 2) * n
                for j in range(0, n, w5):
                    nc.tensor.matmul(pfull[:, base + j:base + j + w5],
                                     lhsT=xnat[:, t, :],
                                     rhs=stall[:, t, j:j + w5],
                                     start=True, stop=True)
                if t == 1:
                    nc.vector.tensor_copy(zw, pfull)
                elif t % 2 == 1:
                    nc.vector.tensor_add(zw, zw, pfull)
            # merge halves + fold in 1/s: znorm = (zl + zr) * stile
            znorm = sg.tile([P, n], F32, tag="yT")   # reuses yT slot
            nc.vector.tensor_add(znorm, zw[:, 0:n], zw[:, n:2 * n])
            nc.vector.tensor_mul(znorm, znorm, stile)

            if L == 4:
                nc.sync.dma_start(dbg_d[:], znorm)
                continue

            # ---- h1T = relu(w1^T Znorm + xT) ----
            h1t = sg.tile([P, n], F32, tag="h1t")
            for j in range(0, n, w5):
                nc.tensor.matmul(pfull[:, j:j + w5], lhsT=w1_sb,
                                 rhs=znorm[:, j:j + w5],
                                 start=True, stop=True)
            nc.vector.tensor_add(h1t, pfull[:, 0:n], xT)
            nc.vector.tensor_relu(h1t, h1t)
            if L == 5:
                nc.sync.dma_start(dbg_d[:], h1t)
                continue

            # ---- tail: out = relu(r0 @ h1 @ w2 / s0 + h1[0, :]) ----
            r0wide = sg.tile([P, 2 * n], F32, tag="zw", name="r0wide")  # zw dead
            r0tile = r0wide[:, 0:n]
            nc.gpsimd.partition_broadcast(r0tile, r0row)
            if L == 6:
                nc.sync.dma_start(dbg_d[:], r0tile)
                continue
            wsum = scr.tile([P, n], F32, tag="scr", name="wsum")
            v = st.tile([P, 1], F32, tag="v")
            # (tensor_tensor_reduce hard-crashes the exec unit on this
            # rig - NRT_EXEC_UNIT_UNRECOVERABLE - so mul + reduce.)
            nc.vector.tensor_mul(wsum, h1t, r0tile)
            nc.vector.tensor_reduce(v, wsum, axis=AX.X, op=ALU.add)
            # o2 = w2^T v as a [128, 1] partition column, so the h1 row-0
            # residual (h1t[:, 0:1], same orientation) adds directly.
            nc.tensor.matmul(pfull[:, n:n + 1], lhsT=w2_sb, rhs=v,
                             start=True, stop=True)
            # drain + normalize by 1/s[0] (stile col 0, all partitions)
            o2n = st.tile([P, 1], F32, tag="o2n")
            nc.vector.tensor_scalar_mul(o2n, pfull[:, n:n + 1], stile[:, 0:1])
            fin = st.tile([P, 1], F32, tag="fin")
            nc.vector.tensor_add(fin, o2n, h1t[:, 0:1])
            nc.vector.tensor_scalar_max(fin, fin, 0.0)
            if L == 8:
                nc.sync.dma_start(dbg_d[:, 0:1], fin)
                continue
            # DRAM side carries the transpose: SBUF APs must keep the
            # partition dim first (moving it to a free dim reads garbage).
            nc.sync.dma_start(out_d[:].rearrange("o p -> p o"), fin)

    nc.compile()
    return nc


_CACHE = {}


def kernel(x, w1, w2, wr):
    x = np.ascontiguousarray(np.asarray(x), dtype=np.float32)
    w1 = np.ascontiguousarray(np.asarray(w1), dtype=np.float32)
    w2 = np.ascontiguousarray(np.asarray(w2), dtype=np.float32)
    wr = np.ascontiguousarray(np.asarray(wr), dtype=np.float32)
    b, n, d = x.shape
    if "nc" not in _CACHE:
        _CACHE["nc"] = build_kernel(n)
    nc = _CACHE["nc"]
    wall = np.ascontiguousarray(np.concatenate([wr, w1, w2], axis=0))
    in_maps = [{"x": x[i], "wall": wall} for i in range(b)]
    res = run_bass_kernel_spmd(nc, in_maps, core_ids=list(range(b)))
    return np.stack([res.results[i]["out"][0] for i in range(b)])
